# revision 1
# baseline (speedup 1.0000x reference)
"""Trainium2 Bass kernel for nn_Attention_10771777978404 (sparse_attention).

Head-parallel (tensor parallel) sharding over 8 NeuronCores:
  - each core owns NH/8 = 2 heads: computes its q/k/v projections (columns of
    wq/wk/wv), RoPE, causal attention with the low-rank sigmoid gate, and the
    per-head attention outputs (transposed, [d, tok]).
  - the rank-32 adapter (gate) weights are replicated; each core computes the
    full [S, S]-gate implicitly, tile by tile, fused into the attention loop.
  - per-head outputs are AllGathered (bf16) across cores; each core then
    computes a 256-column slice of the final `out @ wo.T` (row-sharded wo) and
    the host concatenates the 8 output slices.

Everything on-device is bf16 with fp32 PSUM accumulation.

self-contained: hardcodes the problem shapes; only needs `concourse` (on
PYTHONPATH in this container) + jax axon devices.
"""

import math
from dataclasses import dataclass

import numpy as np
import ml_dtypes

import concourse.bass as bass
import concourse.tile as tile
from concourse import bacc
from concourse import mybir
from concourse import bass_utils
from concourse.tile_rust import add_dep_helper

BF16 = mybir.dt.bfloat16
F32 = mybir.dt.float32
AF = mybir.ActivationFunctionType


@dataclass(frozen=True)
class Cfg:
    B: int = 2
    S: int = 2048
    DIM: int = 2048
    NH: int = 16
    HD: int = 128
    RANK: int = 32
    NCORES: int = 8
    QT: int = 512   # query block (free dim of score tiles)
    KT: int = 128   # key block (partition dim of score tiles)

    @property
    def HLOC(self):
        return self.NH // self.NCORES

    @property
    def DH(self):
        return self.HLOC * self.HD  # per-core head-dim span

    @property
    def KTILES(self):
        return self.DIM // 128  # contraction tiles for projections

    @property
    def QTN(self):
        return self.S // self.QT

    @property
    def DIAG(self):
        return self.QT // self.KT  # k-tiles per diagonal band


FULL = Cfg()


def build_nc(cfg: Cfg = FULL, *, use_gate=True, use_rs=True, use_bcast=True,
             use_deps=True, use_recip=True, use_rope=True, use_mask=True):
    c = cfg
    assert c.HD == 128 and c.KT == 128
    nc = bacc.Bacc("TRN2", target_bir_lowering=False, debug=False,
                   num_devices=c.NCORES)

    # ---- kernel I/O ----
    xT = nc.dram_tensor("xT", [c.B, c.DIM, c.S], BF16, kind="ExternalInput")
    wqT = nc.dram_tensor("wqT", [c.DIM, c.DH], BF16, kind="ExternalInput")
    wkT = nc.dram_tensor("wkT", [c.DIM, c.DH], BF16, kind="ExternalInput")
    wvT = nc.dram_tensor("wvT", [c.DIM, c.DH], BF16, kind="ExternalInput")
    # woc^T[d_local, j]: this core's head-rows of wo^T (= wo column slice), so
    # the core emits a full-size PARTIAL of the output projection from its own
    # heads; the host sums the partials across cores (no device collective).
    wocT = nc.dram_tensor("wocT", [c.DH, c.DIM], BF16, kind="ExternalInput")
    waT = nc.dram_tensor("waT", [c.DIM, 2 * c.RANK], BF16, kind="ExternalInput")
    c2d = nc.dram_tensor("c2d", [c.HD, c.S], BF16, kind="ExternalInput")
    s2d = nc.dram_tensor("s2d", [c.HD, c.S], BF16, kind="ExternalInput")
    pswapd = nc.dram_tensor("pswapd", [c.HD, c.HD], BF16, kind="ExternalInput")
    maskdd = nc.dram_tensor("maskdd", [c.DIAG, c.KT, c.QT], BF16, kind="ExternalInput")

    # partial output projection, transposed: pout[j, b*S + t]
    pout = nc.dram_tensor("pout", [c.DIM, c.B * c.S], F32, kind="ExternalOutput")

    # scratch for broadcasting 1/rowsum across partitions (DRAM round-trip)
    rrd = nc.dram_tensor("rrd", [c.B * c.S // c.QT * c.HLOC, c.QT], F32)
    # gate tiles sigmoid(A')[k, q] staged via DRAM so the scalar engine never
    # alternates between the Sigmoid and Exp function tables (1.3us reload)
    TBLK = c.DIAG * c.QTN * (c.QTN + 1) // 2
    gdram = nc.dram_tensor("gdram", [c.B, TBLK, c.KT, c.QT], BF16)

    isqrt = 1.0 / math.sqrt(c.HD)
    NQC = c.DH // 128          # per-core q/k head chunks (= HLOC)

    from contextlib import ExitStack
    with ExitStack() as _ctx:
        tc = _ctx.enter_context(tile.TileContext(nc))
        cst = _ctx.enter_context(tc.tile_pool(name="const", bufs=1))
        xtp = _ctx.enter_context(tc.tile_pool(name="xt", bufs=1))
        qkp = _ctx.enter_context(tc.tile_pool(name="qk", bufs=1))
        vp = _ctx.enter_context(tc.tile_pool(name="vp", bufs=1))
        adp = _ctx.enter_context(tc.tile_pool(name="ap", bufs=1))
        rtp = _ctx.enter_context(tc.tile_pool(name="rope_t", bufs=1))
        gio = _ctx.enter_context(tc.tile_pool(name="gio", bufs=8))
        pge = _ctx.enter_context(tc.tile_pool(name="pge", bufs=6))
        nrm = _ctx.enter_context(tc.tile_pool(name="norm", bufs=1))
        wop = _ctx.enter_context(tc.tile_pool(name="wo_out", bufs=3))
        pp = _ctx.enter_context(tc.tile_pool(name="pp", bufs=2, space="PSUM"))
        psp = _ctx.enter_context(tc.tile_pool(name="ps", bufs=2, space="PSUM"))
        pgp = pp  # gate psum shares the projection/wo psum pool (bank budget)
        pop = _ctx.enter_context(tc.tile_pool(name="po", bufs=2, space="PSUM"))
        prsp = _ctx.enter_context(tc.tile_pool(name="prs", bufs=2, space="PSUM"))
        if True:
            # ---- constants / weights ----
            wq_sb = cst.tile([128, c.KTILES, c.DH], BF16, name="wq_sb")
            wk_sb = cst.tile([128, c.KTILES, c.DH], BF16, name="wk_sb")
            wv_sb = cst.tile([128, c.KTILES, c.DH], BF16, name="wv_sb")
            woc_sb = cst.tile([128, NQC, c.DIM], BF16, name="woc_sb")
            wa_sb = cst.tile([128, c.KTILES, 2 * c.RANK], BF16, name="wa_sb")
            c2_sb = cst.tile([128, c.S], BF16, name="c2_sb")
            s2_sb = cst.tile([128, c.S], BF16, name="s2_sb")
            psw_sb = cst.tile([128, 128], BF16, name="psw_sb")
            mask_sb = cst.tile([128, c.DIAG, c.QT], BF16, name="mask_sb")
            ones_sb = cst.tile([128, 1], BF16, name="ones_sb")

            for w_sb, w_d in ((wq_sb, wqT), (wk_sb, wkT), (wv_sb, wvT)):
                wr = w_d.ap().rearrange("(t p) m -> p t m", p=128)
                for half in range(2):
                    h0 = half * (c.KTILES // 2)
                    nc.sync.dma_start(out=w_sb[:, h0:h0 + c.KTILES // 2, :],
                                      in_=wr[:, h0:h0 + c.KTILES // 2, :])
            wcr = wocT.ap().rearrange("(h p) j -> p h j", p=128)
            for h in range(NQC):
                nc.sync.dma_start(out=woc_sb[:, h, :], in_=wcr[:, h, :])
            nc.sync.dma_start(out=wa_sb, in_=waT.ap().rearrange("(t p) m -> p t m", p=128))
            nc.sync.dma_start(out=c2_sb, in_=c2d.ap())
            nc.sync.dma_start(out=s2_sb, in_=s2d.ap())
            nc.sync.dma_start(out=psw_sb, in_=pswapd.ap())
            nc.sync.dma_start(out=mask_sb, in_=maskdd.ap().rearrange("j p q -> p j q"))
            nc.vector.memset(ones_sb, 1.0)

            last_exp_inst = None
            for b in range(c.B):
                # ---- load x^T for this batch ----
                xt_sb = xtp.tile([128, c.KTILES, c.S], BF16, name="xt_sb", tag="xt")
                xr = xT.ap()[b].rearrange("(t p) n -> p t n", p=128)
                for kt in range(c.KTILES):
                    nc.sync.dma_start(out=xt_sb[:, kt, :], in_=xr[:, kt, :])

                # ---- projections ----
                # adapters first: the gate-phase sigmoids only need aq/ak, so
                # ACT gets work early while the PE grinds through q/k/v
                aq_sb = adp.tile([32, c.S], BF16, name="aq_sb", tag="aq")
                ak_sb = adp.tile([32, c.S], BF16, name="ak_sb", tag="ak")
                for dst, col0 in ((aq_sb, 0), (ak_sb, c.RANK)):
                    for qt in range(c.QTN):
                        psum = pp.tile([c.RANK, c.QT], F32, name="psum_a", tag="pp")
                        for kt in range(c.KTILES):
                            nc.tensor.matmul(
                                psum[:, :],
                                wa_sb[:, kt, col0:col0 + c.RANK],
                                xt_sb[:, kt, qt * c.QT:(qt + 1) * c.QT],
                                start=(kt == 0), stop=(kt == c.KTILES - 1))
                        nc.vector.tensor_copy(dst[:, qt * c.QT:(qt + 1) * c.QT],
                                              psum[:, :])

                # gate tiles: one Sigmoid run per batch on ACT, staged through
                # DRAM (Exp and Sigmoid live in different ACT tables; each
                # switch costs a ~1.3us reload, so sigmoids and exps are kept
                # in separate runs via explicit deps). Emitted right after the
                # adapter projections so ACT has work during q/k/v.
                last_sig_inst = None
                if use_gate:
                    for qt in range(c.QTN):
                        qsl = slice(qt * c.QT, (qt + 1) * c.QT)
                        for kt in range(c.DIAG * (qt + 1)):
                            ksl = slice(kt * c.KT, (kt + 1) * c.KT)
                            off = (qt * (qt + 1) // 2) * c.DIAG + kt
                            pga = pgp.tile([128, c.QT], F32, name="pga", tag="pp")
                            nc.tensor.matmul(pga[:, :], ak_sb[:, ksl], aq_sb[:, qsl],
                                             start=True, stop=True)
                            gout = gio.tile([128, c.QT], BF16, name="gout", tag="gout")
                            sig = nc.scalar.activation(gout[:, :], pga[:, :], AF.Sigmoid)
                            last_sig_inst = sig.ins
                            nc.sync.dma_start(out=gdram.ap()[b, off], in_=gout[:, :])

                # q^T, k^T: [d, tok] per head chunk; stationary = weight tile
                q_sb = [qkp.tile([128, c.S], BF16, name=f"q{h}_sb", tag=f"q{h}")
                        for h in range(NQC)]
                k_sb = [qkp.tile([128, c.S], BF16, name=f"k{h}_sb", tag=f"k{h}")
                        for h in range(NQC)]
                for dst, w in ((q_sb, wq_sb), (k_sb, wk_sb)):
                    for h in range(NQC):
                        for qt in range(c.QTN):
                            psum = pp.tile([128, c.QT], F32, name="psum_qk", tag="pp")
                            for kt in range(c.KTILES):
                                nc.tensor.matmul(
                                    psum[:, :],
                                    w[:, kt, h * 128:(h + 1) * 128],
                                    xt_sb[:, kt, qt * c.QT:(qt + 1) * c.QT],
                                    start=(kt == 0), stop=(kt == c.KTILES - 1))
                            nc.scalar.copy(dst[h][:, qt * c.QT:(qt + 1) * c.QT], psum[:, :])

                # v: [tok, d] natural; stationary = x^T tile
                v_sb = vp.tile([128, c.S // 128, c.DH], BF16, name="v_sb", tag="v")
                for tt in range(c.S // 128):
                    psum = pp.tile([128, c.DH], F32, name="psum_v", tag="pp")
                    for kt in range(c.KTILES):
                        nc.tensor.matmul(
                            psum[:, :],
                            xt_sb[:, kt, tt * 128:(tt + 1) * 128],
                            wv_sb[:, kt, :],
                            start=(kt == 0), stop=(kt == c.KTILES - 1))
                    nc.vector.tensor_copy(v_sb[:, tt, :], psum[:, :])

                # ---- RoPE on q^T / k^T (in place) ----
                # out = t*C2 + swap(t)*S2 ; swap via PE permutation matmul
                for tiles in ((q_sb, k_sb) if use_rope else ()):
                    for h in range(NQC):
                        for qt in range(c.QTN):
                            sl = slice(qt * c.QT, (qt + 1) * c.QT)
                            pswp = pp.tile([128, c.QT], F32, name="pswp", tag="pp")
                            nc.tensor.matmul(pswp[:, :], psw_sb[:, :],
                                             tiles[h][:, sl], start=True, stop=True)
                            m1 = rtp.tile([128, c.QT], BF16, name="rope_m1", tag="m1")
                            m2 = rtp.tile([128, c.QT], BF16, name="rope_m2", tag="m2")
                            nc.vector.tensor_mul(m1[:, :], tiles[h][:, sl], c2_sb[:, sl])
                            nc.vector.tensor_mul(m2[:, :], pswp[:, :], s2_sb[:, sl])
                            nc.vector.tensor_add(tiles[h][:, sl], m1[:, :], m2[:, :])

                # ---- attention (gates + scores + AV + normalize + pout,
                #       interleaved per 512-query block qt) ----
                # normalized per-head outputs og[d, h, tok] stay in SBUF for
                # the output-projection partial matmul
                og_sb = nrm.tile([128, c.HLOC, c.S], BF16, name="og_sb", tag="og")
                for qt in range(c.QTN):
                    qsl = slice(qt * c.QT, (qt + 1) * c.QT)
                    nkt = c.DIAG * qt + c.DIAG  # causal k tiles
                    po = [pop.tile([128, c.QT], F32, name=f"po{h}", tag="po")
                          for h in range(c.HLOC)]
                    prs = [prsp.tile([1, c.QT], F32, name=f"prs{h}", tag="prs")
                           for h in range(c.HLOC)]
                    for kt in range(nkt):
                        ksl = slice(kt * c.KT, (kt + 1) * c.KT)
                        off = (qt * (qt + 1) // 2) * c.DIAG + kt
                        gin = gio.tile([128, c.QT], BF16, name="gin", tag="gin")
                        if use_gate:
                            nc.sync.dma_start(out=gin[:, :], in_=gdram.ap()[b, off])
                        else:
                            nc.vector.memset(gin[:, :], 1.0)
                        for h in range(c.HLOC):
                            ps = psp.tile([128, c.QT], F32, name="ps", tag="ps")
                            nc.tensor.matmul(ps[:, :], k_sb[h][:, ksl],
                                             q_sb[h][:, qsl], start=True, stop=True)
                            p_sb = pge.tile([128, c.QT], BF16, name="p_sb", tag="p")
                            ex = nc.scalar.activation(p_sb[:, :], ps[:, :], AF.Exp,
                                                      scale=isqrt)
                            if use_deps and use_gate and (qt, kt, h) == (0, 0, 0):
                                add_dep_helper(ex.ins, last_sig_inst,
                                               reason="ACT table: exps after this batch's sigmoids")
                            last_exp_inst = ex.ins
                            j = kt - c.DIAG * qt
                            if j >= 0 and use_mask:
                                # diagonal band: causal 0/1 mask applied AFTER
                                # exp (exp(s-1e9)=0 == exp(s)*0); bf16 2x-mode
                                # multiply is cheaper than the fp32 PSUM add
                                pm = pge.tile([128, c.QT], BF16, name="pm", tag="pm")
                                nc.vector.tensor_mul(pm[:, :], p_sb[:, :],
                                                     mask_sb[:, j, :])
                                p_sb = pm
                            # rowsum (pre-gate) via ones-vector matmul
                            if use_rs:
                                nc.tensor.matmul(prs[h][:, :], ones_sb[:, :],
                                                 p_sb[:, :],
                                                 start=(kt == 0), stop=(kt == nkt - 1))
                            pgm = pge.tile([128, c.QT], BF16, name="pgm", tag="pgm")
                            nc.vector.tensor_mul(pgm[:, :], p_sb[:, :], gin[:, :])
                            # out_h^T[d, q] += v[k,d].T @ p_gated[k,q]
                            nc.tensor.matmul(po[h][:, :],
                                             v_sb[:, kt, h * 128:(h + 1) * 128],
                                             pgm[:, :],
                                             start=(kt == 0), stop=(kt == nkt - 1))
                    # normalize: og = po * (1/rowsum) broadcast over partitions
                    for h in range(c.HLOC):
                        ouq = nrm.tile([128, c.QT], F32, name="ouq", tag="ouq")
                        nc.scalar.copy(ouq[:, :], po[h][:, :])
                        rbc = nrm.tile([128, c.QT], F32, name="rbc", tag="rbc")
                        if use_rs and use_bcast:
                            rs = nrm.tile([1, c.QT], F32, name="rs", tag="rs")
                            nc.scalar.copy(rs[:, :], prs[h][:, :])
                            rr = nrm.tile([1, c.QT], F32, name="rr", tag="rr")
                            if use_recip:
                                nc.vector.reciprocal_approx_fast(out=rr[:, :], in_=rs[:, :])
                            else:
                                nc.vector.tensor_copy(rr[:, :], rs[:, :])
                            ridx = (b * c.QTN + qt) * c.HLOC + h
                            rrow = rrd.ap()[ridx:ridx + 1, :]
                            nc.sync.dma_start(out=rrow, in_=rr[:, :])
                            nc.sync.dma_start(
                                out=rbc[:, :],
                                in_=bass.AP(tensor=rrd.ap().tensor, offset=ridx * c.QT,
                                            ap=[[0, 128], [1, c.QT]]))
                        else:
                            nc.vector.memset(rbc[:, :], 1.0)
                        nc.vector.tensor_mul(og_sb[:, h, qsl], ouq[:, :], rbc[:, :])

                    # ---- output-projection partial for this query block:
                    # pout[:, qt] = woc^T.T @ og[:, :, qt] — emitted per qt so
                    # it overlaps the next qt's attention on the PE
                    for ch in range(c.DIM // 128):
                        pf = pp.tile([128, c.QT], F32, name="pf", tag="pp")
                        for h in range(c.HLOC):
                            nc.tensor.matmul(
                                pf[:, :],
                                woc_sb[:, h, ch * 128:(ch + 1) * 128],
                                og_sb[:, h, qsl],
                                start=(h == 0), stop=(h == c.HLOC - 1))
                        f_sb = wop.tile([128, c.QT], F32, name="f_sb", tag="f")
                        nc.vector.tensor_copy(f_sb[:, :], pf[:, :])
                        nc.sync.dma_start(
                            out=pout.ap()[ch * 128:(ch + 1) * 128,
                                          b * c.S + qt * c.QT: b * c.S + (qt + 1) * c.QT],
                            in_=f_sb[:, :])

    nc.compile()
    return nc


def make_core_inputs(inputs: dict, cfg: Cfg = FULL):
    """Host-side sharding: returns in_maps (one dict per core)."""
    c = cfg
    bf16 = ml_dtypes.bfloat16
    x = np.asarray(inputs["x"])
    mask = np.asarray(inputs["mask"])
    fc = np.asarray(inputs["freqs_cos"])
    fs = np.asarray(inputs["freqs_sin"])
    wq, wk, wv, wo = (np.asarray(inputs[k]) for k in ("wq", "wk", "wv", "wo"))
    wa_q, wa_k = np.asarray(inputs["wa_q"]), np.asarray(inputs["wa_k"])

    xT = np.ascontiguousarray(x.transpose(0, 2, 1)).astype(bf16)
    waT = np.ascontiguousarray(np.concatenate([wa_q, wa_k], axis=0).T).astype(bf16)

    # rope tables in [d, tok] layout
    c2 = np.empty((c.HD, c.S), np.float32)
    s2 = np.empty((c.HD, c.S), np.float32)
    c2[0::2] = fc.T
    c2[1::2] = fc.T
    s2[0::2] = -fs.T
    s2[1::2] = fs.T
    c2 = c2.astype(bf16)
    s2 = s2.astype(bf16)

    psw = np.zeros((c.HD, c.HD), np.float32)
    idx = np.arange(c.HD)
    psw[idx, idx ^ 1] = 1.0
    psw = psw.astype(bf16)

    # diagonal-band mask patterns [j][k, q], extracted from the input mask
    qt_last = c.QTN - 1
    q0 = qt_last * c.QT
    maskd = np.empty((c.DIAG, c.KT, c.QT), np.float32)
    for j in range(c.DIAG):
        k0 = (c.DIAG * qt_last + j) * c.KT
        # multiplicative 0/1 form: positions the additive mask leaves at 0
        # (unmasked) become 1.0, masked positions (-1e9) become 0.0
        maskd[j] = (mask[0, 0, q0:q0 + c.QT, k0:k0 + c.KT].T == 0.0)
    maskd = maskd.astype(bf16)

    in_maps = []
    for ci in range(c.NCORES):
        rows = slice(ci * c.DH, (ci + 1) * c.DH)
        in_maps.append({
            "xT": xT,
            "wqT": np.ascontiguousarray(wq[rows].T).astype(bf16),
            "wkT": np.ascontiguousarray(wk[rows].T).astype(bf16),
            "wvT": np.ascontiguousarray(wv[rows].T).astype(bf16),
            "wocT": np.ascontiguousarray(wo[:, rows].T).astype(bf16),
            "waT": waT,
            "c2d": c2,
            "s2d": s2,
            "pswapd": psw,
            "maskdd": maskd,
        })
    return in_maps


def assemble_output(results, cfg: Cfg = FULL) -> np.ndarray:
    c = cfg
    total = np.zeros((c.DIM, c.B * c.S), np.float32)
    for ci in range(c.NCORES):
        total += np.asarray(results[ci]["pout"])
    return np.ascontiguousarray(
        total.reshape(c.DIM, c.B, c.S).transpose(1, 2, 0))


_NC_CACHE = {}


def run(nc, in_maps, trace=False, cfg: Cfg = FULL, **kw):
    return bass_utils.run_bass_kernel_spmd(
        nc, in_maps, core_ids=list(range(cfg.NCORES)), trace=trace, **kw)


def kernel(**inputs) -> np.ndarray:
    cfg = FULL
    if cfg not in _NC_CACHE:
        _NC_CACHE[cfg] = build_nc(cfg)
    nc = _NC_CACHE[cfg]
    in_maps = make_core_inputs(inputs, cfg)
    res = run(nc, in_maps, cfg=cfg)
    return assemble_output(res.results, cfg)


if __name__ == "__main__":
    nc = build_nc(FULL)
    print("built ok")



# revision 16
# speedup vs baseline: 110.5288x; 110.5288x over previous
"""Trainium2 Bass kernel for nn_Attention_10771777978404 (sparse_attention).

Sharding over 8 NeuronCores: 2 batch-groups x 4 cores (tensor parallel over
heads within each batch group).
  - core ci handles batch ci//4 and heads [4*(ci%4), 4*(ci%4)+4): it computes
    its q/k/v projections (columns of wq/wk/wv), RoPE, causal attention with
    the low-rank sigmoid gate, and a full-width partial of the output
    projection from its 4 heads (rows of wo).
  - the rank-32 adapter weights are replicated inside each batch group; the
    sigmoid gate is computed as 1/(1+exp(-A)) so the scalar engine only ever
    uses the Exp table (no Sigmoid-table reloads, no DRAM staging).
  - host sums the 4 partial output projections per batch (fp16 partials).

Everything on-device is bf16 with fp32 PSUM accumulation.

self-contained: hardcodes the problem shapes; only needs `concourse` (on
PYTHONPATH in this container) + jax axon devices.
"""

import math
from contextlib import ExitStack
from dataclasses import dataclass

import numpy as np
import ml_dtypes

import concourse.bass as bass
import concourse.tile as tile
from concourse import bacc
from concourse import mybir
from concourse import bass_utils

BF16 = mybir.dt.bfloat16
F16 = mybir.dt.float16
F32 = mybir.dt.float32
AF = mybir.ActivationFunctionType


@dataclass(frozen=True)
class Cfg:
    B: int = 2
    S: int = 2048
    DIM: int = 2048
    NH: int = 16
    HD: int = 128
    RANK: int = 32
    NCORES: int = 8
    GROUPS: int = 2     # batch groups of 4 cores
    QT: int = 512       # query block (free dim of score tiles)
    KT: int = 128       # key block (partition dim of score tiles)
    repeat: int = 1     # hardware-loop repetitions of the whole body (timing)

    @property
    def CPG(self):
        return self.NCORES // self.GROUPS  # cores per batch group

    @property
    def HLOC(self):
        return self.NH // self.CPG  # heads per core (4)

    @property
    def DH(self):
        return self.HLOC * self.HD  # per-core head-dim span (512)

    @property
    def KTILES(self):
        return self.DIM // 128  # contraction tiles for projections

    @property
    def QTN(self):
        return self.S // self.QT

    @property
    def DIAG(self):
        return self.QT // self.KT  # k-tiles per diagonal band


FULL = Cfg()


def build_nc(cfg: Cfg = FULL):
    c = cfg
    assert c.HD == 128 and c.KT == 128
    nc = bacc.Bacc("TRN2", target_bir_lowering=False, debug=False,
                   num_devices=c.NCORES)

    # ---- kernel I/O (per core: one batch, HLOC heads) ----
    xT = nc.dram_tensor("xT", [c.DIM, c.S], BF16, kind="ExternalInput")
    wqT = nc.dram_tensor("wqT", [c.DIM, c.DH], BF16, kind="ExternalInput")
    wkT = nc.dram_tensor("wkT", [c.DIM, c.DH], BF16, kind="ExternalInput")
    wvT = nc.dram_tensor("wvT", [c.DIM, c.DH], BF16, kind="ExternalInput")
    wocT = nc.dram_tensor("wocT", [c.DH, c.DIM], BF16, kind="ExternalInput")
    waT = nc.dram_tensor("waT", [c.DIM, 2 * c.RANK], BF16, kind="ExternalInput")
    c2d = nc.dram_tensor("c2d", [c.HD, c.S], BF16, kind="ExternalInput")
    s2d = nc.dram_tensor("s2d", [c.HD, c.S], BF16, kind="ExternalInput")
    pswapd = nc.dram_tensor("pswapd", [c.HD, c.HD], BF16, kind="ExternalInput")
    maskdd = nc.dram_tensor("maskdd", [c.DIAG, c.KT, c.QT], BF16, kind="ExternalInput")

    # partial output projection, transposed: pout[j, t] (fp16; host sums the
    # 4 partials of each batch group in fp32)
    pout = nc.dram_tensor("pout", [c.DIM, c.S], F16, kind="ExternalOutput")

    isqrt = 1.0 / math.sqrt(c.HD)

    with ExitStack() as _ctx:
        tc = _ctx.enter_context(tile.TileContext(nc))
        # persistent pools (whole-iteration lifetime)
        cst = _ctx.enter_context(tc.tile_pool(name="const", bufs=1))
        adp = _ctx.enter_context(tc.tile_pool(name="ap", bufs=1))
        qkp = _ctx.enter_context(tc.tile_pool(name="qk", bufs=1))
        vp = _ctx.enter_context(tc.tile_pool(name="vp", bufs=1))
        rtp = _ctx.enter_context(tc.tile_pool(name="rope_t", bufs=1))
        pge = _ctx.enter_context(tc.tile_pool(name="pge", bufs=4))
        gwk = _ctx.enter_context(tc.tile_pool(name="gwk", bufs=2))
        ogp = _ctx.enter_context(tc.tile_pool(name="og", bufs=1))
        wop = _ctx.enter_context(tc.tile_pool(name="wo_out", bufs=2))
        # PSUM pools (8 banks total): pp 2 + ps/pga/rbc 3 + po 2 + prs 1
        pp = _ctx.enter_context(tc.tile_pool(name="pp", bufs=2, space="PSUM"))
        psp = _ctx.enter_context(tc.tile_pool(name="ps", bufs=3, space="PSUM"))
        pop = _ctx.enter_context(tc.tile_pool(name="po", bufs=2, space="PSUM"))
        prsp = _ctx.enter_context(tc.tile_pool(name="prs", bufs=1, space="PSUM"))

        def body():
            # ---- constants ----
            c2_sb = cst.tile([128, c.S], BF16, name="c2_sb", tag="c2")
            s2_sb = cst.tile([128, c.S], BF16, name="s2_sb", tag="s2")
            psw_sb = cst.tile([128, 128], BF16, name="psw_sb", tag="psw")
            mask_sb = cst.tile([128, c.DIAG, c.QT], BF16, name="mask_sb", tag="mask")
            ones_sb = cst.tile([128, 1], BF16, name="ones_sb", tag="ones")
            oner_sb = cst.tile([1, 128], F16, name="oner_sb", tag="oner")

            aq_sb = adp.tile([c.RANK, c.S], BF16, name="aq_sb", tag="aq")
            ak_sb = adp.tile([c.RANK, c.S], BF16, name="ak_sb", tag="ak")
            q_sb = [qkp.tile([128, c.S], BF16, name=f"q{h}_sb", tag=f"q{h}")
                    for h in range(c.HLOC)]
            k_sb = [qkp.tile([128, c.S], BF16, name=f"k{h}_sb", tag=f"k{h}")
                    for h in range(c.HLOC)]
            v_sb = vp.tile([128, c.S // 128, c.DH], BF16, name="v_sb", tag="v")

            with tc.tile_pool(name="xtp", bufs=1) as xtp:
                wa_sb = xtp.tile([128, c.KTILES, 2 * c.RANK], BF16, name="wa_sb", tag="wa")
                nc.sync.dma_start(out=wa_sb,
                                  in_=waT.ap().rearrange("(t p) m -> p t m", p=128))
                # x^T chunks early so the adapter matmuls can start ASAP
                xt_sb = xtp.tile([128, c.KTILES, c.S], BF16, name="xt_sb", tag="xt")
                xr = xT.ap().rearrange("(t p) n -> p t n", p=128)
                for kt in range(c.KTILES):
                    nc.sync.dma_start(out=xt_sb[:, kt, :], in_=xr[:, kt, :])
                nc.sync.dma_start(out=c2_sb, in_=c2d.ap())
                nc.sync.dma_start(out=s2_sb, in_=s2d.ap())
                nc.sync.dma_start(out=psw_sb, in_=pswapd.ap())
                nc.sync.dma_start(out=mask_sb,
                                  in_=maskdd.ap().rearrange("j p q -> p j q"))
                nc.vector.memset(ones_sb, 1.0)
                nc.vector.memset(oner_sb, 1.0)

                # ---- adapter projections: aq, ak [RANK, S] ----
                # kt-outer with 8 live accumulators (borrowed across the four
                # PSUM pools) so the PE issues 8 matmuls per arriving x chunk
                # instead of idling at DMA pace
                combos = [(dst, col0, qt)
                          for dst, col0 in ((aq_sb, 0), (ak_sb, c.RANK))
                          for qt in range(c.QTN)]
                apools = [pp, pp, psp, psp, psp, pop, pop, prsp]
                acc = [apools[i].tile([c.RANK, c.QT], F32, name=f"acc_a{i}",
                                      tag=("pp", "pp", "ps", "ps", "ps", "po",
                                           "po", "prs")[i])
                       for i in range(8)]
                for kt in range(c.KTILES):
                    for i, (dst, col0, qt) in enumerate(combos):
                        nc.tensor.matmul(
                            acc[i][:, :],
                            wa_sb[:, kt, col0:col0 + c.RANK],
                            xt_sb[:, kt, qt * c.QT:(qt + 1) * c.QT],
                            start=(kt == 0), stop=(kt == c.KTILES - 1))
                for i, (dst, col0, qt) in enumerate(combos):
                    nc.vector.tensor_copy(dst[:, qt * c.QT:(qt + 1) * c.QT],
                                          acc[i][:, :])

                # ---- q/k projections ----
                with tc.tile_pool(name="wqk", bufs=1) as wp:
                    wq_sb = wp.tile([128, c.KTILES, c.DH], BF16, name="wq_sb", tag="wq")
                    wk_sb = wp.tile([128, c.KTILES, c.DH], BF16, name="wk_sb", tag="wk")
                    for w_sb, w_d in ((wq_sb, wqT), (wk_sb, wkT)):
                        wr = w_d.ap().rearrange("(t p) m -> p t m", p=128)
                        for half in range(2):
                            h0 = half * (c.KTILES // 2)
                            nc.sync.dma_start(out=w_sb[:, h0:h0 + c.KTILES // 2, :],
                                              in_=wr[:, h0:h0 + c.KTILES // 2, :])
                    for dst, w in ((q_sb, wq_sb), (k_sb, wk_sb)):
                        for h in range(c.HLOC):
                            for qt in range(c.QTN):
                                psum = pp.tile([128, c.QT], F32, name="psum_qk", tag="pp")
                                for kt in range(c.KTILES):
                                    nc.tensor.matmul(
                                        psum[:, :],
                                        w[:, kt, h * 128:(h + 1) * 128],
                                        xt_sb[:, kt, qt * c.QT:(qt + 1) * c.QT],
                                        start=(kt == 0), stop=(kt == c.KTILES - 1))
                                nc.scalar.copy(dst[h][:, qt * c.QT:(qt + 1) * c.QT],
                                               psum[:, :])

                # ---- v projection: [tok, d] natural, moving 512 wide ----
                # RoPE tiles are interleaved into the v loop: the rope chain
                # is DVE/GPSIMD-paced, the v matmuls keep the PE busy
                rope_tiles = [(tiles, h, qt)
                              for tiles in (q_sb, k_sb)
                              for h in range(c.HLOC)
                              for qt in range(c.QTN)]

                def rope_step(i):
                    # m1 on GPSIMD (SBUF-only engine), m2 on DVE (PSUM read),
                    # final add alternates so neither engine is the pacer
                    tiles, h, qt = rope_tiles[i]
                    eng = nc.vector if i % 2 == 0 else nc.gpsimd
                    sl = slice(qt * c.QT, (qt + 1) * c.QT)
                    pswp = psp.tile([128, c.QT], F32, name="pswp", tag="ps")
                    nc.tensor.matmul(pswp[:, :], psw_sb[:, :],
                                     tiles[h][:, sl], start=True, stop=True)
                    m1 = rtp.tile([128, c.QT], BF16, name="rope_m1",
                                  tag=f"m1{i % 2}")
                    m2 = rtp.tile([128, c.QT], BF16, name="rope_m2",
                                  tag=f"m2{i % 2}")
                    nc.gpsimd.tensor_mul(m1[:, :], tiles[h][:, sl], c2_sb[:, sl])
                    nc.vector.tensor_mul(m2[:, :], pswp[:, :], s2_sb[:, sl])
                    eng.tensor_add(tiles[h][:, sl], m1[:, :], m2[:, :])

                with tc.tile_pool(name="wvp", bufs=1) as wvpool:
                    wv_sb = wvpool.tile([128, c.KTILES, c.DH], BF16, name="wv_sb", tag="wv")
                    wr = wvT.ap().rearrange("(t p) m -> p t m", p=128)
                    for half in range(2):
                        h0 = half * (c.KTILES // 2)
                        nc.sync.dma_start(out=wv_sb[:, h0:h0 + c.KTILES // 2, :],
                                          in_=wr[:, h0:h0 + c.KTILES // 2, :])
                    for tt in range(c.S // 128):
                        psum = pp.tile([128, c.DH], F32, name="psum_v", tag="pp")
                        for kt in range(c.KTILES):
                            nc.tensor.matmul(
                                psum[:, :],
                                xt_sb[:, kt, tt * 128:(tt + 1) * 128],
                                wv_sb[:, kt, :],
                                start=(kt == 0), stop=(kt == c.KTILES - 1))
                        nc.scalar.copy(v_sb[:, tt, :], psum[:, :])
                        rope_step(2 * tt)
                        rope_step(2 * tt + 1)

            # ---- attention + output projection, per 512-query block ----
            with tc.tile_pool(name="wog", bufs=1) as wog:
                woc_sb = wog.tile([128, c.HLOC, c.DIM], BF16, name="woc_sb", tag="woc")
                wcr = wocT.ap().rearrange("(h p) j -> p h j", p=128)
                for h in range(c.HLOC):
                    nc.sync.dma_start(out=woc_sb[:, h, :], in_=wcr[:, h, :])
                # double-buffered per-qt gate tiles r = sigmoid(A)[k, q]
                rg_sb = [wog.tile([128, c.DIAG * c.QTN, c.QT], BF16,
                                  name=f"rg{i}_sb", tag=f"rg{i}") for i in range(2)]

                def gate_step(qt, kt):
                    # r = 1/(1+exp(-A)) via the (shared) Exp table
                    qsl = slice(qt * c.QT, (qt + 1) * c.QT)
                    ksl = slice(kt * c.KT, (kt + 1) * c.KT)
                    pga = psp.tile([128, c.QT], F32, name="pga", tag="ps")
                    nc.tensor.matmul(pga[:, :], ak_sb[:, ksl], aq_sb[:, qsl],
                                     start=True, stop=True)
                    ge = gwk.tile([128, c.QT], BF16, name="ge", tag="ge")
                    nc.scalar.activation(ge[:, :], pga[:, :], AF.Exp, scale=-1.0)
                    gt = gwk.tile([128, c.QT], F32, name="gt", tag="gt")
                    nc.vector.tensor_scalar_add(gt[:, :], ge[:, :], 1.0)
                    gr = gwk.tile([128, c.QT], F32, name="gr", tag="gr")
                    nc.vector.reciprocal_approx_fast(out=gr[:, :], in_=gt[:, :])
                    nc.gpsimd.tensor_copy(rg_sb[qt % 2][:, kt, :], gr[:, :])

                for kt in range(c.DIAG):
                    gate_step(0, kt)

                for qt in range(c.QTN):
                    qsl = slice(qt * c.QT, (qt + 1) * c.QT)
                    nkt = c.DIAG * (qt + 1)  # causal k tiles
                    rg = rg_sb[qt % 2]

                    og_sb = ogp.tile([128, c.HLOC, c.QT], BF16, name="og_sb", tag="og")
                    # normalize chain of head h is emitted early in head h+1's
                    # score phase so its latency hides behind score matmuls
                    pending_norm = [None]

                    def make_normalize(h, po, prs):
                        def norm():
                            rr = gwk.tile([1, c.QT], F32, name="rr", tag="rr")
                            nc.vector.reciprocal_approx_fast(out=rr[:, :],
                                                             in_=prs[:, :])
                            rrh = gwk.tile([1, c.QT], F16, name="rrh", tag="rrh")
                            nc.vector.tensor_copy(rrh[:, :], rr[:, :])
                            rbc = psp.tile([128, c.QT], F32, name="rbc", tag="ps")
                            nc.tensor.matmul(rbc[:, :], oner_sb[:, :], rrh[:, :],
                                             start=True, stop=True)
                            # DVE can't take two PSUM operands; stage the
                            # broadcast in SBUF via ACT
                            rbs = gwk.tile([128, c.QT], F16, name="rbs", tag="rbs")
                            nc.scalar.copy(rbs[:, :], rbc[:, :])
                            nc.vector.tensor_mul(og_sb[:, h, :], po[:, :],
                                                 rbs[:, :])
                        return norm

                    for h in range(c.HLOC):
                        po = pop.tile([128, c.QT], F32, name="po", tag="po")
                        prs = prsp.tile([1, c.QT], F32, name="prs", tag="prs")

                        # score matmuls run PIPE steps ahead of the dependent
                        # rowsum/AV matmuls so the exp/mask/gate chain latency
                        # stays off the in-order PE queue
                        PIPE = 2
                        stage = []  # (kt, p_or_pm_tile, pgm_tile)

                        def score_step(kt):
                            ksl = slice(kt * c.KT, (kt + 1) * c.KT)
                            ps = psp.tile([128, c.QT], F32, name="ps", tag="ps")
                            nc.tensor.matmul(ps[:, :], k_sb[h][:, ksl],
                                             q_sb[h][:, qsl], start=True, stop=True)
                            p_sb = pge.tile([128, c.QT], BF16, name="p_sb", tag="p")
                            nc.scalar.activation(p_sb[:, :], ps[:, :], AF.Exp,
                                                 scale=isqrt)
                            j = kt - c.DIAG * qt
                            if j >= 0:
                                # diagonal band: 0/1 causal mask after exp
                                pm = pge.tile([128, c.QT], BF16, name="pm", tag="pm")
                                nc.vector.tensor_mul(pm[:, :], p_sb[:, :],
                                                     mask_sb[:, j, :])
                                p_sb = pm
                            pgm = pge.tile([128, c.QT], BF16, name="pgm", tag="pgm")
                            nc.vector.tensor_mul(pgm[:, :], p_sb[:, :], rg[:, kt, :])
                            stage.append((kt, p_sb, pgm))

                        def drain_step():
                            kt, p_sb, pgm = stage.pop(0)
                            # pre-gate rowsum (softmax denominator)
                            nc.tensor.matmul(prs[:, :], ones_sb[:, :], p_sb[:, :],
                                             start=(kt == 0), stop=(kt == nkt - 1))
                            # out_h^T[d, q] += v[k, d].T @ p_gated[k, q]
                            nc.tensor.matmul(po[:, :],
                                             v_sb[:, kt, h * 128:(h + 1) * 128],
                                             pgm[:, :],
                                             start=(kt == 0), stop=(kt == nkt - 1))

                        for kt in range(nkt):
                            score_step(kt)
                            if kt == 1 and pending_norm[0] is not None:
                                pending_norm[0]()
                                pending_norm[0] = None
                            if len(stage) > PIPE:
                                drain_step()
                        while stage:
                            drain_step()
                        pending_norm[0] = make_normalize(h, po, prs)
                    # last head's normalize must land before the wo matmuls
                    pending_norm[0]()

                    # output-projection partial for this query block,
                    # interleaved with the NEXT block's gate generation so the
                    # PE fills the gate chain's latency with wo matmuls
                    nkt2 = c.DIAG * (qt + 2) if qt + 1 < c.QTN else 0
                    for ch in range(c.DIM // 128):
                        pf = pp.tile([128, c.QT], F32, name="pf", tag="pp")
                        for h in range(c.HLOC):
                            nc.tensor.matmul(
                                pf[:, :],
                                woc_sb[:, h, ch * 128:(ch + 1) * 128],
                                og_sb[:, h, :],
                                start=(h == 0), stop=(h == c.HLOC - 1))
                        f_sb = wop.tile([128, c.QT], F16, name="f_sb", tag="f")
                        nc.scalar.copy(f_sb[:, :], pf[:, :])
                        nc.sync.dma_start(
                            out=pout.ap()[ch * 128:(ch + 1) * 128, qsl],
                            in_=f_sb[:, :])
                        if ch < nkt2:
                            gate_step(qt + 1, ch)

        if c.repeat > 1:
            with tc.For_i(0, c.repeat, 1):
                body()
        else:
            body()

    nc.compile()
    return nc


def make_core_inputs(inputs: dict, cfg: Cfg = FULL):
    """Host-side sharding: returns in_maps (one dict per core)."""
    c = cfg
    bf16 = ml_dtypes.bfloat16
    x = np.asarray(inputs["x"])
    mask = np.asarray(inputs["mask"])
    fc = np.asarray(inputs["freqs_cos"])
    fs = np.asarray(inputs["freqs_sin"])
    wq, wk, wv, wo = (np.asarray(inputs[k]) for k in ("wq", "wk", "wv", "wo"))
    wa_q, wa_k = np.asarray(inputs["wa_q"]), np.asarray(inputs["wa_k"])

    xTb = [np.ascontiguousarray(x[b].T).astype(bf16) for b in range(c.B)]
    waT = np.ascontiguousarray(np.concatenate([wa_q, wa_k], axis=0).T).astype(bf16)

    # rope tables in [d, tok] layout
    c2 = np.empty((c.HD, c.S), np.float32)
    s2 = np.empty((c.HD, c.S), np.float32)
    c2[0::2] = fc.T
    c2[1::2] = fc.T
    s2[0::2] = -fs.T
    s2[1::2] = fs.T
    c2 = c2.astype(bf16)
    s2 = s2.astype(bf16)

    psw = np.zeros((c.HD, c.HD), np.float32)
    idx = np.arange(c.HD)
    psw[idx, idx ^ 1] = 1.0
    psw = psw.astype(bf16)

    # diagonal-band mask patterns [j][k, q], extracted from the input mask
    qt_last = c.QTN - 1
    q0 = qt_last * c.QT
    maskd = np.empty((c.DIAG, c.KT, c.QT), np.float32)
    for j in range(c.DIAG):
        k0 = (c.DIAG * qt_last + j) * c.KT
        maskd[j] = (mask[0, 0, q0:q0 + c.QT, k0:k0 + c.KT].T == 0.0)
    maskd = maskd.astype(bf16)

    wslices = []
    for hs in range(c.CPG):
        rows = slice(hs * c.DH, (hs + 1) * c.DH)
        wslices.append({
            "wqT": np.ascontiguousarray(wq[rows].T).astype(bf16),
            "wkT": np.ascontiguousarray(wk[rows].T).astype(bf16),
            "wvT": np.ascontiguousarray(wv[rows].T).astype(bf16),
            "wocT": np.ascontiguousarray(wo[:, rows].T).astype(bf16),
        })

    in_maps = []
    for ci in range(c.NCORES):
        b = ci // c.CPG
        hs = ci % c.CPG
        in_maps.append({
            "xT": xTb[b],
            **wslices[hs],
            "waT": waT,
            "c2d": c2,
            "s2d": s2,
            "pswapd": psw,
            "maskdd": maskd,
        })
    return in_maps


def assemble_output(results, cfg: Cfg = FULL) -> np.ndarray:
    c = cfg
    out = np.empty((c.B, c.S, c.DIM), np.float32)
    for b in range(c.B):
        total = np.zeros((c.DIM, c.S), np.float32)
        for hs in range(c.CPG):
            total += np.asarray(results[b * c.CPG + hs]["pout"]).astype(np.float32)
        out[b] = total.T
    return out


_NC_CACHE = {}


def run(nc, in_maps, trace=False, cfg: Cfg = FULL, **kw):
    return bass_utils.run_bass_kernel_spmd(
        nc, in_maps, core_ids=list(range(cfg.NCORES)), trace=trace, **kw)


def kernel(**inputs) -> np.ndarray:
    cfg = FULL
    if cfg not in _NC_CACHE:
        _NC_CACHE[cfg] = build_nc(cfg)
    nc = _NC_CACHE[cfg]
    in_maps = make_core_inputs(inputs, cfg)
    res = run(nc, in_maps, cfg=cfg)
    return assemble_output(res.results, cfg)


if __name__ == "__main__":
    nc = build_nc(FULL)
    print("built ok")


# revision 27
# speedup vs baseline: 120.2805x; 1.0882x over previous
"""Trainium2 Bass kernel for nn_Attention_10771777978404 (sparse_attention).

Sharding over 8 NeuronCores: 2 batch-groups x 4 cores (tensor parallel over
heads within each batch group).
  - core ci handles batch ci//4 and heads [4*(ci%4), 4*(ci%4)+4): it computes
    its q/k/v projections (columns of wq/wk/wv), RoPE, causal attention with
    the low-rank sigmoid gate, and a full-width partial of the output
    projection from its 4 heads (rows of wo).
  - the rank-32 adapter weights are replicated inside each batch group; the
    sigmoid gate is computed as 1/(1+exp(-A)) so the scalar engine only ever
    uses the Exp table (no Sigmoid-table reloads, no DRAM staging).
  - host sums the 4 partial output projections per batch (fp16 partials).

Everything on-device is bf16 with fp32 PSUM accumulation.

self-contained: hardcodes the problem shapes; only needs `concourse` (on
PYTHONPATH in this container) + jax axon devices.
"""

import math
from contextlib import ExitStack
from dataclasses import dataclass

import numpy as np
import ml_dtypes

import concourse.bass as bass
import concourse.tile as tile
from concourse import bacc
from concourse import mybir
from concourse import bass_utils

BF16 = mybir.dt.bfloat16
F8 = mybir.dt.float8e4
DR = mybir.MatmulPerfMode.DoubleRow
WSCALE = 32.0  # fp8 weight prescale (keeps weights out of the subnormal range)
F16 = mybir.dt.float16
F32 = mybir.dt.float32
AF = mybir.ActivationFunctionType


@dataclass(frozen=True)
class Cfg:
    B: int = 2
    S: int = 2048
    DIM: int = 2048
    NH: int = 16
    HD: int = 128
    RANK: int = 32
    NCORES: int = 8
    GROUPS: int = 2     # batch groups of 4 cores
    QT: int = 512       # query block (free dim of score tiles)
    KT: int = 128       # key block (partition dim of score tiles)
    repeat: int = 1     # hardware-loop repetitions of the whole body (timing)
    # ablation flags (profiling on hardware; all True for the real kernel)
    use_gate: bool = True
    use_rowsum: bool = True
    use_attn: bool = True
    use_wo: bool = True
    # fp8e4m3 DoubleRow projections: ~1.6x faster projections in the sim but
    # costs 8e-2 relative error (peaked softmax amplifies logit noise) vs the
    # 2e-2 gate — keep False
    use_fp8: bool = False

    @property
    def CPG(self):
        return self.NCORES // self.GROUPS  # cores per batch group

    @property
    def HLOC(self):
        return self.NH // self.CPG  # heads per core (4)

    @property
    def DH(self):
        return self.HLOC * self.HD  # per-core head-dim span (512)

    @property
    def KTILES(self):
        return self.DIM // 128  # contraction tiles for projections

    @property
    def QTN(self):
        return self.S // self.QT

    @property
    def DIAG(self):
        return self.QT // self.KT  # k-tiles per diagonal band

    @property
    def KP(self):
        return self.KTILES // 2  # DoubleRow contraction pairs


FULL = Cfg()


def build_nc(cfg: Cfg = FULL):
    c = cfg
    assert c.HD == 128 and c.KT == 128
    nc = bacc.Bacc("TRN2", target_bir_lowering=False, debug=False,
                   num_devices=c.NCORES)

    # ---- kernel I/O (per core: one batch, HLOC heads) ----
    PDT = F8 if c.use_fp8 else BF16  # projection operand dtype
    xT = nc.dram_tensor("xT", [c.DIM, c.S], PDT, kind="ExternalInput")
    wqT = nc.dram_tensor("wqT", [c.DIM, c.DH], PDT, kind="ExternalInput")
    wkT = nc.dram_tensor("wkT", [c.DIM, c.DH], PDT, kind="ExternalInput")
    wvT = nc.dram_tensor("wvT", [c.DIM, c.DH], PDT, kind="ExternalInput")
    wocT = nc.dram_tensor("wocT", [c.DH, c.DIM], BF16, kind="ExternalInput")
    waT = nc.dram_tensor("waT", [c.DIM, 2 * c.RANK], PDT, kind="ExternalInput")
    c2d = nc.dram_tensor("c2d", [c.HD, c.S], BF16, kind="ExternalInput")
    s2d = nc.dram_tensor("s2d", [c.HD, c.S], BF16, kind="ExternalInput")
    pswapd = nc.dram_tensor("pswapd", [c.HD, c.HD], BF16, kind="ExternalInput")
    maskdd = nc.dram_tensor("maskdd", [c.DIAG, c.KT, c.QT], BF16, kind="ExternalInput")

    # partial output projection, transposed: pout[j, t] (fp16; host sums the
    # 4 partials of each batch group in fp32)
    pout = nc.dram_tensor("pout", [c.DIM, c.S], F16, kind="ExternalOutput")

    isqrt = 1.0 / math.sqrt(c.HD)
    # fp8 path: q,k,v,aq,ak all carry a WSCALE factor from the prescaled
    # weights; compensate in the exp scales (scores, gate) and on the host (v)
    wsq = WSCALE * WSCALE if c.use_fp8 else 1.0
    sc_score = isqrt / wsq
    sc_gate = -1.0 / wsq
    NKT = c.KP if c.use_fp8 else c.KTILES
    pmode = DR if c.use_fp8 else None

    with ExitStack() as _ctx:
        tc = _ctx.enter_context(tile.TileContext(nc))
        # persistent pools (whole-iteration lifetime)
        cst = _ctx.enter_context(tc.tile_pool(name="const", bufs=1))
        adp = _ctx.enter_context(tc.tile_pool(name="ap", bufs=1))
        qkp = _ctx.enter_context(tc.tile_pool(name="qk", bufs=1))
        vp = _ctx.enter_context(tc.tile_pool(name="vp", bufs=1))
        rtp = _ctx.enter_context(tc.tile_pool(name="rope_t", bufs=1))
        pge = _ctx.enter_context(tc.tile_pool(name="pge", bufs=5))
        gwk = _ctx.enter_context(tc.tile_pool(name="gwk", bufs=2))
        ogp = _ctx.enter_context(tc.tile_pool(name="og", bufs=1))
        wop = _ctx.enter_context(tc.tile_pool(name="wo_out", bufs=2))
        # PSUM pools (8 banks total): pp 2 + ps/pga/rbc 3 + po 2 + prs 1
        pp = _ctx.enter_context(tc.tile_pool(name="pp", bufs=2, space="PSUM"))
        psp = _ctx.enter_context(tc.tile_pool(name="ps", bufs=3, space="PSUM"))
        pop = _ctx.enter_context(tc.tile_pool(name="po", bufs=2, space="PSUM"))
        prsp = _ctx.enter_context(tc.tile_pool(name="prs", bufs=1, space="PSUM"))

        def body():
            # ---- constants ----
            c2_sb = cst.tile([128, c.S], BF16, name="c2_sb", tag="c2")
            s2_sb = cst.tile([128, c.S], BF16, name="s2_sb", tag="s2")
            psw_sb = cst.tile([128, 128], BF16, name="psw_sb", tag="psw")
            mask_sb = cst.tile([128, c.DIAG, c.QT], BF16, name="mask_sb", tag="mask")
            ones_sb = cst.tile([128, 1], BF16, name="ones_sb", tag="ones")
            oner_sb = cst.tile([1, 128], F16, name="oner_sb", tag="oner")

            aq_sb = adp.tile([c.RANK, c.S], BF16, name="aq_sb", tag="aq")
            ak_sb = adp.tile([c.RANK, c.S], BF16, name="ak_sb", tag="ak")
            q_sb = [qkp.tile([128, c.S], BF16, name=f"q{h}_sb", tag=f"q{h}")
                    for h in range(c.HLOC)]
            k_sb = [qkp.tile([128, c.S], BF16, name=f"k{h}_sb", tag=f"k{h}")
                    for h in range(c.HLOC)]
            v_sb = vp.tile([128, c.S // 128, c.DH], BF16, name="v_sb", tag="v")

            with tc.tile_pool(name="xtp", bufs=1) as xtp:
                if c.use_fp8:
                    wa_sb = xtp.tile([128, NKT, 2, 2 * c.RANK], F8,
                                     name="wa_sb", tag="wa")
                    nc.sync.dma_start(
                        out=wa_sb,
                        in_=waT.ap().rearrange("(t two p) m -> p t two m",
                                               p=128, two=2))
                    xt_sb = xtp.tile([128, NKT, 2, c.S], F8, name="xt_sb", tag="xt")
                    xr = xT.ap().rearrange("(t two p) n -> p t two n", p=128, two=2)
                    for kt in range(NKT):
                        nc.sync.dma_start(out=xt_sb[:, kt, :, :], in_=xr[:, kt, :, :])
                else:
                    wa_sb = xtp.tile([128, c.KTILES, 2 * c.RANK], BF16,
                                     name="wa_sb", tag="wa")
                    nc.sync.dma_start(out=wa_sb,
                                      in_=waT.ap().rearrange("(t p) m -> p t m", p=128))
                    xt_sb = xtp.tile([128, c.KTILES, c.S], BF16, name="xt_sb", tag="xt")
                    xr = xT.ap().rearrange("(t p) n -> p t n", p=128)
                    for kt in range(c.KTILES):
                        nc.sync.dma_start(out=xt_sb[:, kt, :], in_=xr[:, kt, :])

                def xsl(j, sl):
                    return xt_sb[:, j, :, sl] if c.use_fp8 else xt_sb[:, j, sl]

                def wsl(w, j, sl):
                    return w[:, j, :, sl] if c.use_fp8 else w[:, j, sl]
                nc.sync.dma_start(out=c2_sb, in_=c2d.ap())
                nc.sync.dma_start(out=s2_sb, in_=s2d.ap())
                nc.sync.dma_start(out=psw_sb, in_=pswapd.ap())
                nc.sync.dma_start(out=mask_sb,
                                  in_=maskdd.ap().rearrange("j p q -> p j q"))
                nc.vector.memset(ones_sb, 1.0)
                nc.vector.memset(oner_sb, 1.0)

                # ---- adapter projections: aq, ak [RANK, S] ----
                # kt-outer with 8 live accumulators (borrowed across the four
                # PSUM pools) so the PE issues 8 matmuls per arriving x chunk
                # instead of idling at DMA pace
                combos = [(dst, col0, qt)
                          for dst, col0 in ((aq_sb, 0), (ak_sb, c.RANK))
                          for qt in range(c.QTN)]
                apools = [pp, pp, psp, psp, psp, pop, pop, prsp]
                acc = [apools[i].tile([c.RANK, c.QT], F32, name=f"acc_a{i}",
                                      tag=("pp", "pp", "ps", "ps", "ps", "po",
                                           "po", "prs")[i])
                       for i in range(8)]
                for kt in range(NKT):
                    for i, (dst, col0, qt) in enumerate(combos):
                        nc.tensor.matmul(
                            acc[i][:, :],
                            wsl(wa_sb, kt, slice(col0, col0 + c.RANK)),
                            xsl(kt, slice(qt * c.QT, (qt + 1) * c.QT)),
                            start=(kt == 0), stop=(kt == NKT - 1),
                            perf_mode=pmode)
                for i, (dst, col0, qt) in enumerate(combos):
                    nc.vector.tensor_copy(dst[:, qt * c.QT:(qt + 1) * c.QT],
                                          acc[i][:, :])

                # ---- q/k projections ----
                with tc.tile_pool(name="wqk", bufs=1) as wp:
                    if c.use_fp8:
                        wq_sb = wp.tile([128, NKT, 2, c.DH], F8, name="wq_sb", tag="wq")
                        wk_sb = wp.tile([128, NKT, 2, c.DH], F8, name="wk_sb", tag="wk")
                        for w_sb, w_d in ((wq_sb, wqT), (wk_sb, wkT)):
                            nc.sync.dma_start(
                                out=w_sb,
                                in_=w_d.ap().rearrange("(t two p) m -> p t two m",
                                                       p=128, two=2))
                    else:
                        wq_sb = wp.tile([128, c.KTILES, c.DH], BF16, name="wq_sb", tag="wq")
                        wk_sb = wp.tile([128, c.KTILES, c.DH], BF16, name="wk_sb", tag="wk")
                        for w_sb, w_d in ((wq_sb, wqT), (wk_sb, wkT)):
                            wr = w_d.ap().rearrange("(t p) m -> p t m", p=128)
                            for half in range(2):
                                h0 = half * (c.KTILES // 2)
                                nc.sync.dma_start(out=w_sb[:, h0:h0 + c.KTILES // 2, :],
                                                  in_=wr[:, h0:h0 + c.KTILES // 2, :])
                    for dst, w in ((q_sb, wq_sb), (k_sb, wk_sb)):
                        for h in range(c.HLOC):
                            for qt in range(c.QTN):
                                psum = pp.tile([128, c.QT], F32, name="psum_qk", tag="pp")
                                for kt in range(NKT):
                                    nc.tensor.matmul(
                                        psum[:, :],
                                        wsl(w, kt, slice(h * 128, (h + 1) * 128)),
                                        xsl(kt, slice(qt * c.QT, (qt + 1) * c.QT)),
                                        start=(kt == 0), stop=(kt == NKT - 1),
                                        perf_mode=pmode)
                                nc.scalar.copy(dst[h][:, qt * c.QT:(qt + 1) * c.QT],
                                               psum[:, :])

                # ---- v projection: [tok, d] natural, moving 512 wide ----
                # RoPE tiles are interleaved into the v loop: the rope chain
                # is DVE/GPSIMD-paced, the v matmuls keep the PE busy
                rope_tiles = [(tiles, h, qt)
                              for tiles in (q_sb, k_sb)
                              for h in range(c.HLOC)
                              for qt in range(c.QTN)]

                def rope_step(i):
                    # m1 on GPSIMD (SBUF-only engine), m2 on DVE (PSUM read),
                    # final add alternates so neither engine is the pacer
                    tiles, h, qt = rope_tiles[i]
                    eng = nc.vector if i % 2 == 0 else nc.gpsimd
                    sl = slice(qt * c.QT, (qt + 1) * c.QT)
                    pswp = psp.tile([128, c.QT], F32, name="pswp", tag="ps")
                    nc.tensor.matmul(pswp[:, :], psw_sb[:, :],
                                     tiles[h][:, sl], start=True, stop=True)
                    m1 = rtp.tile([128, c.QT], BF16, name="rope_m1",
                                  tag=f"m1{i % 2}")
                    m2 = rtp.tile([128, c.QT], BF16, name="rope_m2",
                                  tag=f"m2{i % 2}")
                    nc.gpsimd.tensor_mul(m1[:, :], tiles[h][:, sl], c2_sb[:, sl])
                    nc.vector.tensor_mul(m2[:, :], pswp[:, :], s2_sb[:, sl])
                    eng.tensor_add(tiles[h][:, sl], m1[:, :], m2[:, :])

                with tc.tile_pool(name="wvp", bufs=1) as wvpool:
                    if c.use_fp8:
                        wv_sb = wvpool.tile([128, NKT, 2, c.DH], F8,
                                            name="wv_sb", tag="wv")
                        nc.sync.dma_start(
                            out=wv_sb,
                            in_=wvT.ap().rearrange("(t two p) m -> p t two m",
                                                   p=128, two=2))
                    else:
                        wv_sb = wvpool.tile([128, c.KTILES, c.DH], BF16,
                                            name="wv_sb", tag="wv")
                        wr = wvT.ap().rearrange("(t p) m -> p t m", p=128)
                        for half in range(2):
                            h0 = half * (c.KTILES // 2)
                            nc.sync.dma_start(out=wv_sb[:, h0:h0 + c.KTILES // 2, :],
                                              in_=wr[:, h0:h0 + c.KTILES // 2, :])
                    for tt in range(c.S // 128):
                        psum = pp.tile([128, c.DH], F32, name="psum_v", tag="pp")
                        for kt in range(NKT):
                            nc.tensor.matmul(
                                psum[:, :],
                                xsl(kt, slice(tt * 128, (tt + 1) * 128)),
                                wsl(wv_sb, kt, slice(0, c.DH)),
                                start=(kt == 0), stop=(kt == NKT - 1),
                                perf_mode=pmode)
                        nc.scalar.copy(v_sb[:, tt, :], psum[:, :])
                        rope_step(2 * tt)
                        rope_step(2 * tt + 1)

            # ---- attention + output projection, per 512-query block ----
            with tc.tile_pool(name="wog", bufs=1) as wog:
                woc_sb = wog.tile([128, c.HLOC, c.DIM], BF16, name="woc_sb", tag="woc")
                wcr = wocT.ap().rearrange("(h p) j -> p h j", p=128)
                for h in range(c.HLOC):
                    nc.sync.dma_start(out=woc_sb[:, h, :], in_=wcr[:, h, :])
                # double-buffered per-qt gate tiles r = sigmoid(A)[k, q]
                rg_sb = [wog.tile([128, c.DIAG * c.QTN, c.QT], BF16,
                                  name=f"rg{i}_sb", tag=f"rg{i}") for i in range(2)]

                def gate_step(qt, kt):
                    # r = 1/(1+exp(-A)) via the (shared) Exp table
                    qsl = slice(qt * c.QT, (qt + 1) * c.QT)
                    ksl = slice(kt * c.KT, (kt + 1) * c.KT)
                    pga = psp.tile([128, c.QT], F32, name="pga", tag="ps")
                    nc.tensor.matmul(pga[:, :], ak_sb[:, ksl], aq_sb[:, qsl],
                                     start=True, stop=True)
                    ge = pge.tile([128, c.QT], BF16, name="ge", tag="p")
                    nc.scalar.activation(ge[:, :], pga[:, :], AF.Exp, scale=sc_gate)
                    gt = gwk.tile([128, c.QT], F32, name="gt", tag="gt")
                    nc.vector.tensor_scalar_add(gt[:, :], ge[:, :], 1.0)
                    gr = gwk.tile([128, c.QT], F32, name="gr", tag="gr")
                    nc.vector.reciprocal_approx_fast(out=gr[:, :], in_=gt[:, :])
                    nc.gpsimd.tensor_copy(rg_sb[qt % 2][:, kt, :], gr[:, :])

                if c.use_gate and c.use_attn:
                    for kt in range(c.DIAG):
                        gate_step(0, kt)

                for qt in range(c.QTN):
                    qsl = slice(qt * c.QT, (qt + 1) * c.QT)
                    nkt = c.DIAG * (qt + 1)  # causal k tiles
                    rg = rg_sb[qt % 2]

                    og_sb = ogp.tile([128, c.HLOC, c.QT], BF16, name="og_sb", tag="og")
                    if not c.use_attn:
                        nc.vector.memset(og_sb, 0.0)
                    # normalize chain of head h is emitted early in head h+1's
                    # score phase so its latency hides behind score matmuls
                    pending_norm = [None]

                    def make_normalize(h, po, prs):
                        def norm():
                            if not c.use_rowsum:
                                nc.vector.tensor_copy(og_sb[:, h, :], po[:, :])
                                return
                            rr = gwk.tile([1, c.QT], F32, name="rr", tag="rr")
                            nc.vector.reciprocal_approx_fast(out=rr[:, :],
                                                             in_=prs[:, :])
                            rrh = gwk.tile([1, c.QT], F16, name="rrh", tag="rrh")
                            nc.vector.tensor_copy(rrh[:, :], rr[:, :])
                            rbc = psp.tile([128, c.QT], F32, name="rbc", tag="ps")
                            nc.tensor.matmul(rbc[:, :], oner_sb[:, :], rrh[:, :],
                                             start=True, stop=True)
                            # DVE can't take two PSUM operands; stage the
                            # broadcast in SBUF via ACT
                            rbs = gwk.tile([128, c.QT], F16, name="rbs", tag="rbs")
                            nc.scalar.copy(rbs[:, :], rbc[:, :])
                            nc.vector.tensor_mul(og_sb[:, h, :], po[:, :],
                                                 rbs[:, :])
                        return norm

                    for h in range(c.HLOC if c.use_attn else 0):
                        po = pop.tile([128, c.QT], F32, name="po", tag="po")
                        prs = prsp.tile([1, c.QT], F32, name="prs", tag="prs")

                        # score matmuls run PIPE steps ahead of the dependent
                        # rowsum/AV matmuls so the exp/mask/gate chain latency
                        # stays off the in-order PE queue
                        PIPE = 3
                        stage = []  # (kt, p_or_pm_tile, pgm_tile)

                        def score_step(kt):
                            ksl = slice(kt * c.KT, (kt + 1) * c.KT)
                            ps = psp.tile([128, c.QT], F32, name="ps", tag="ps")
                            nc.tensor.matmul(ps[:, :], k_sb[h][:, ksl],
                                             q_sb[h][:, qsl], start=True, stop=True)
                            p_sb = pge.tile([128, c.QT], BF16, name="p_sb", tag="p")
                            nc.scalar.activation(p_sb[:, :], ps[:, :], AF.Exp,
                                                 scale=sc_score)
                            j = kt - c.DIAG * qt
                            if j >= 0:
                                # diagonal band: 0/1 causal mask after exp
                                pm = pge.tile([128, c.QT], BF16, name="pm", tag="pm")
                                nc.vector.tensor_mul(pm[:, :], p_sb[:, :],
                                                     mask_sb[:, j, :])
                                p_sb = pm
                            if c.use_gate:
                                pgm = pge.tile([128, c.QT], BF16, name="pgm",
                                               tag="pgm")
                                nc.vector.tensor_mul(pgm[:, :], p_sb[:, :],
                                                     rg[:, kt, :])
                            else:
                                pgm = p_sb
                            stage.append((kt, p_sb, pgm))

                        def drain_step():
                            kt, p_sb, pgm = stage.pop(0)
                            # pre-gate rowsum (softmax denominator)
                            if c.use_rowsum:
                                nc.tensor.matmul(prs[:, :], ones_sb[:, :],
                                                 p_sb[:, :],
                                                 start=(kt == 0),
                                                 stop=(kt == nkt - 1))
                            # out_h^T[d, q] += v[k, d].T @ p_gated[k, q]
                            nc.tensor.matmul(po[:, :],
                                             v_sb[:, kt, h * 128:(h + 1) * 128],
                                             pgm[:, :],
                                             start=(kt == 0), stop=(kt == nkt - 1))

                        for kt in range(nkt):
                            score_step(kt)
                            if kt == 1 and pending_norm[0] is not None:
                                pending_norm[0]()
                                pending_norm[0] = None
                            if len(stage) > PIPE:
                                drain_step()
                        while stage:
                            drain_step()
                        pending_norm[0] = make_normalize(h, po, prs)
                    # last head's normalize must land before the wo matmuls
                    if pending_norm[0] is not None:
                        pending_norm[0]()

                    # output-projection partial for this query block,
                    # interleaved with the NEXT block's gate generation so the
                    # PE fills the gate chain's latency with wo matmuls
                    nkt2 = (c.DIAG * (qt + 2)
                            if (qt + 1 < c.QTN and c.use_gate and c.use_attn)
                            else 0)
                    for ch in range(c.DIM // 128 if c.use_wo else 0):
                        pf = pp.tile([128, c.QT], F32, name="pf", tag="pp")
                        for h in range(c.HLOC):
                            nc.tensor.matmul(
                                pf[:, :],
                                woc_sb[:, h, ch * 128:(ch + 1) * 128],
                                og_sb[:, h, :],
                                start=(h == 0), stop=(h == c.HLOC - 1))
                        f_sb = wop.tile([128, c.QT], F16, name="f_sb", tag="f")
                        nc.scalar.copy(f_sb[:, :], pf[:, :])
                        nc.sync.dma_start(
                            out=pout.ap()[ch * 128:(ch + 1) * 128, qsl],
                            in_=f_sb[:, :])
                        if ch < nkt2:
                            gate_step(qt + 1, ch)
                    if not c.use_wo:
                        for kt in range(nkt2):
                            gate_step(qt + 1, kt)

        if c.repeat > 1:
            with tc.For_i(0, c.repeat, 1):
                body()
        else:
            body()

    nc.compile()
    return nc


def make_core_inputs(inputs: dict, cfg: Cfg = FULL):
    """Host-side sharding: returns in_maps (one dict per core)."""
    c = cfg
    bf16 = ml_dtypes.bfloat16
    x = np.asarray(inputs["x"])
    mask = np.asarray(inputs["mask"])
    fc = np.asarray(inputs["freqs_cos"])
    fs = np.asarray(inputs["freqs_sin"])
    wq, wk, wv, wo = (np.asarray(inputs[k]) for k in ("wq", "wk", "wv", "wo"))
    wa_q, wa_k = np.asarray(inputs["wa_q"]), np.asarray(inputs["wa_k"])

    import concourse.mybir as _mb
    pdt = _mb.dt.np(F8) if c.use_fp8 else bf16
    wsc = WSCALE if c.use_fp8 else 1.0
    xTb = [np.ascontiguousarray(x[b].T).astype(pdt) for b in range(c.B)]
    waT = np.ascontiguousarray(
        np.concatenate([wa_q, wa_k], axis=0).T * wsc).astype(pdt)

    # rope tables in [d, tok] layout
    c2 = np.empty((c.HD, c.S), np.float32)
    s2 = np.empty((c.HD, c.S), np.float32)
    c2[0::2] = fc.T
    c2[1::2] = fc.T
    s2[0::2] = -fs.T
    s2[1::2] = fs.T
    c2 = c2.astype(bf16)
    s2 = s2.astype(bf16)

    psw = np.zeros((c.HD, c.HD), np.float32)
    idx = np.arange(c.HD)
    psw[idx, idx ^ 1] = 1.0
    psw = psw.astype(bf16)

    # diagonal-band mask patterns [j][k, q], extracted from the input mask
    qt_last = c.QTN - 1
    q0 = qt_last * c.QT
    maskd = np.empty((c.DIAG, c.KT, c.QT), np.float32)
    for j in range(c.DIAG):
        k0 = (c.DIAG * qt_last + j) * c.KT
        maskd[j] = (mask[0, 0, q0:q0 + c.QT, k0:k0 + c.KT].T == 0.0)
    maskd = maskd.astype(bf16)

    wslices = []
    for hs in range(c.CPG):
        rows = slice(hs * c.DH, (hs + 1) * c.DH)
        wslices.append({
            "wqT": np.ascontiguousarray(wq[rows].T * wsc).astype(pdt),
            "wkT": np.ascontiguousarray(wk[rows].T * wsc).astype(pdt),
            "wvT": np.ascontiguousarray(wv[rows].T * wsc).astype(pdt),
            "wocT": np.ascontiguousarray(wo[:, rows].T).astype(bf16),
        })

    in_maps = []
    for ci in range(c.NCORES):
        b = ci // c.CPG
        hs = ci % c.CPG
        in_maps.append({
            "xT": xTb[b],
            **wslices[hs],
            "waT": waT,
            "c2d": c2,
            "s2d": s2,
            "pswapd": psw,
            "maskdd": maskd,
        })
    return in_maps


def assemble_output(results, cfg: Cfg = FULL) -> np.ndarray:
    c = cfg
    out = np.empty((c.B, c.S, c.DIM), np.float32)
    inv = 1.0 / (WSCALE if c.use_fp8 else 1.0)
    for b in range(c.B):
        total = np.zeros((c.DIM, c.S), np.float32)
        for hs in range(c.CPG):
            total += np.asarray(results[b * c.CPG + hs]["pout"]).astype(np.float32)
        out[b] = total.T * inv
    return out


_NC_CACHE = {}


def run(nc, in_maps, trace=False, cfg: Cfg = FULL, **kw):
    return bass_utils.run_bass_kernel_spmd(
        nc, in_maps, core_ids=list(range(cfg.NCORES)), trace=trace, **kw)


def kernel(**inputs) -> np.ndarray:
    cfg = FULL
    if cfg not in _NC_CACHE:
        _NC_CACHE[cfg] = build_nc(cfg)
    nc = _NC_CACHE[cfg]
    in_maps = make_core_inputs(inputs, cfg)
    res = run(nc, in_maps, cfg=cfg)
    return assemble_output(res.results, cfg)


if __name__ == "__main__":
    nc = build_nc(FULL)
    print("built ok")
